# revision 1
# baseline (speedup 1.0000x reference)
"""Trainium2 Bass kernel for nn_Cross_attention_2 (sparse_attention).

Math (B=1, C=32, D=36, H=W=48, P=9):
  xc = conv1x1(x, W_img, b_img)            # per-voxel channel mix
  v  = unfold(xc)                          # (C, L=1024, 81) non-overlapping 9x9 patches
  px = LeakyReLU(v @ (W2@W1)^T + bias)     # the two Linears collapse to A = W2@W1
  att[c] = px[c] @ py[c]^T / 81            # (C, 1024, 1024)

Sharding: channels C=32 split across 8 cores (4 each). Params replicated
(per-core slices precomputed on host). Each core reads full x, y.

Per-core device pipeline (fp32 data, fp32r PE mode; all matmul outputs at
PSUM partition base 0 — fp32r codegen requires it):
  conv:      3 accumulating zero-padded block-diag matmuls (K=128/128/32)
             -> xc_sb (37, 4, 2304): rows kd*4+o, row 36 = 1.0 (bias row)
  transform: unfold folded into strided rhs APs; 9 kw-accumulation passes per
             output tile; combined weight TM includes channel select + A + bias
  att:       pxT/pyT kept as (81, 1024); out tiles (128, 512) per matmul
"""

import sys

sys.path.insert(0, "/opt/trn_rl_repo")

import contextlib
import os

import numpy as np

import concourse.bass as bass  # noqa: F401
import concourse.tile as tile
from concourse import bacc, mybir
from concourse.bass_utils import run_bass_kernel_spmd

P = 9
P2 = 81
C = 32
D = 36
HWF = 2304
ND = 4  # pd blocks (D/9)
L = 1024
N_CORES = 8
CPC = 4  # channels per core

F32 = mybir.dt.float32
F32R = mybir.dt.float32r

_CACHE = {}
last_results = None  # BassKernelResults of the most recent run (for test.py)

_HW_CHUNKS = [(0, 512), (512, 512), (1024, 512), (1536, 512), (2048, 256)]
_KD_PASSES = [(0, 4), (4, 4), (8, 1)]  # (kd0, nkd) conv passes


def _build():
    if "nc" in _CACHE:
        return _CACHE["nc"]

    nc = bacc.Bacc("TRN2", target_bir_lowering=False, debug=False,
                   num_devices=N_CORES)
    x_d = nc.dram_tensor("x", (C, D, HWF), F32R, kind="ExternalInput").ap()
    y_d = nc.dram_tensor("y", (C, D, HWF), F32R, kind="ExternalInput").ap()
    # wblk: (128, 216) = conv lhsT for (t in 2) x (pass i in 3), 36 cols each
    wblk_d = nc.dram_tensor("wblk", (128, 216), F32R, kind="ExternalInput").ap()
    # tm: (37, 2*4*9*81) combined transform weights in SBUF layout
    tm_d = nc.dram_tensor("tm", (37, 2 * CPC * P * P2), F32R,
                          kind="ExternalInput").ap()
    ones_d = nc.dram_tensor("ones", (1, ND * HWF), F32R,
                            kind="ExternalInput").ap()
    att_d = nc.dram_tensor("att", (CPC, L, L), F32, kind="ExternalOutput").ap()

    with tile.TileContext(nc) as tc:
        with contextlib.ExitStack() as ctx:
            consts = ctx.enter_context(tc.tile_pool(name="consts", bufs=1))
            xbp = ctx.enter_context(tc.tile_pool(name="xb", bufs=3))
            xbp2 = ctx.enter_context(tc.tile_pool(name="xb2", bufs=1))
            tmpp = ctx.enter_context(tc.tile_pool(name="tmp", bufs=2))
            outp = ctx.enter_context(tc.tile_pool(name="outp", bufs=2))
            cps = ctx.enter_context(tc.tile_pool(name="cps", bufs=2, space="PSUM"))
            tps = ctx.enter_context(tc.tile_pool(name="tps", bufs=2, space="PSUM"))
            aps = ctx.enter_context(tc.tile_pool(name="aps", bufs=3, space="PSUM"))

            wb_sb = consts.tile([128, 216], F32R, tag="wb")
            nc.sync.dma_start(out=wb_sb[:, :], in_=wblk_d[:, :])
            tm_sb = consts.tile([37, 2 * CPC * P * P2], F32R, tag="tm")
            nc.sync.dma_start(out=tm_sb[:, :], in_=tm_d[:, :])
            tm_v = tm_sb.rearrange("p (t c kw j) -> p t c kw j", t=2, c=CPC, kw=P)

            xc_sb = []
            px_sb = []
            for t in range(2):
                xt = consts.tile([37, ND, HWF], F32R, tag=f"xc{t}")
                nc.sync.dma_start(
                    out=xt[36:37, :, :],
                    in_=ones_d.rearrange("p (d h) -> p d h", d=ND))
                xc_sb.append(xt)
                px_sb.append([consts.tile([P2, L], F32R, tag=f"px{t}{c}",
                                          name=f"px{t}{c}")
                              for c in range(CPC)])

            for t in range(2):
                src = x_d if t == 0 else y_d
                for pd in range(ND):
                    xbs = []
                    for i, (kd0, nkd) in enumerate(_KD_PASSES):
                        kp = 32 * nkd
                        pool = xbp if nkd == 4 else xbp2
                        xb = pool.tile([kp, HWF], F32R, tag=f"xb{min(i, 1)}",
                                       name=f"xb{min(i, 1)}")
                        rows = src[:, pd * P + kd0: pd * P + kd0 + nkd, :]
                        nc.sync.dma_start(out=xb[:, :],
                                          in_=rows.transpose([1, 0, 2]))
                        xbs.append(xb)
                    for h0, hn in _HW_CHUNKS:
                        ps = cps.tile([36, 512], F32, tag="cps")
                        for i, (kd0, nkd) in enumerate(_KD_PASSES):
                            kp = 32 * nkd
                            lhs = wb_sb[0:kp,
                                        (t * 3 + i) * 36: (t * 3 + i + 1) * 36]
                            nc.tensor.matmul(
                                ps[:, :hn], lhs, xbs[i][:, h0: h0 + hn],
                                start=(i == 0), stop=(i == 2))
                        dst = xc_sb[t][0:36, pd, h0: h0 + hn]
                        if (h0 // 512) % 2 == 0:
                            nc.vector.tensor_copy(out=dst, in_=ps[:, :hn])
                        else:
                            nc.scalar.copy(out=dst, in_=ps[:, :hn])

                # transform: z = sum_kw TM[t,c,kw].T @ xc[:, :, kw::9]
                for c in range(CPC):
                    for ch in range(2):  # l-chunks of 512 (pd pairs)
                        zp = tps.tile([P2, 512], F32, tag="tps")
                        for kw in range(P):
                            rhs = xc_sb[t][:, 2 * ch: 2 * ch + 2, kw:HWF:P]
                            nc.tensor.matmul(
                                zp[:, :], tm_v[:, t, c, kw, :], rhs,
                                start=(kw == 0), stop=(kw == P - 1))
                        # LeakyReLU(z) = max(0.2*z, z)
                        zm = tmpp.tile([P2, 512], F32, tag="zm")
                        nc.scalar.mul(zm[:, :], zp[:, :], 0.2)
                        nc.vector.tensor_tensor(
                            out=px_sb[t][c][:, ch * 512: ch * 512 + 512],
                            in0=zp[:, :], in1=zm[:, :],
                            op=mybir.AluOpType.max)

            # att[c] = pxT[c].T @ pyT[c]
            for c in range(CPC):
                for m in range(8):  # l1 chunks of 128
                    ob = outp.tile([128, L], F32, tag="ob")
                    for nch in range(2):  # l2 chunks of 512
                        ap_ = aps.tile([128, 512], F32, tag="aps")
                        nc.tensor.matmul(
                            ap_[:, :],
                            px_sb[0][c][:, m * 128: m * 128 + 128],
                            px_sb[1][c][:, nch * 512: nch * 512 + 512],
                            start=True, stop=True)
                        dst = ob[:, nch * 512: nch * 512 + 512]
                        if nch % 2 == 0:
                            nc.vector.tensor_copy(out=dst, in_=ap_[:, :])
                        else:
                            nc.scalar.copy(out=dst, in_=ap_[:, :])
                    nc.sync.dma_start(
                        out=att_d[c, m * 128: m * 128 + 128, :], in_=ob[:, :])

    nc.compile()
    _CACHE["nc"] = nc
    return nc


def _host_prep(x, y, W_img, b_img, W_fea, b_fea, W1, W2):
    """Build per-core wblk / tm arrays. Returns in_maps list."""
    x = np.ascontiguousarray(np.asarray(x, np.float32).reshape(C, D, HWF))
    y = np.ascontiguousarray(np.asarray(y, np.float32).reshape(C, D, HWF))
    W_img = np.asarray(W_img, np.float32)
    b_img = np.asarray(b_img, np.float32)
    W_fea = np.asarray(W_fea, np.float32)
    b_fea = np.asarray(b_fea, np.float32)
    A = np.asarray(W2, np.float32) @ np.asarray(W1, np.float32)  # (81, 81)
    rowsum = A.sum(axis=1)  # (81,)
    ones = np.ones((1, ND * HWF), np.float32)

    in_maps = []
    for r in range(N_CORES):
        Wl = [W_img[r * CPC:(r + 1) * CPC, :], W_fea[r * CPC:(r + 1) * CPC, :]]
        bl = [b_img[r * CPC:(r + 1) * CPC], b_fea[r * CPC:(r + 1) * CPC]]

        # conv lhsT: wblk[kd_l*32+c', (t*3+i)*36 + kd*4+o] = W_t[o, c']
        #            with kd = kd0_i + kd_l
        wblk = np.zeros((128, 216), np.float32)
        for t in range(2):
            for i, (kd0, nkd) in enumerate(_KD_PASSES):
                for kd_l in range(nkd):
                    kd = kd0 + kd_l
                    rows = slice(kd_l * 32, kd_l * 32 + 32)
                    for o in range(CPC):
                        col = (t * 3 + i) * 36 + kd * 4 + o
                        wblk[rows, col] = Wl[t][o, :]

        # tm[p, t, c, kw, j]; p = kd*4 + o, row 36 = bias (kw=0 only)
        tm = np.zeros((37, 2, CPC, P, P2), np.float32)
        At = np.stack([A / P2, A])                 # x-side carries the 1/81
        bias = np.stack([np.outer(bl[0], rowsum) / P2,
                         np.outer(bl[1], rowsum)])  # (2, 4, 81)
        for kd in range(P):
            for o in range(CPC):
                p = kd * 4 + o
                # tm[p, t, o, kw, j] = At[t, j, kd*9+kw]
                tm[p, :, o, :, :] = At[:, :, kd * P:(kd + 1) * P].transpose(0, 2, 1)
        tm[36, :, :, 0, :] = bias
        tm = tm.reshape(37, 2 * CPC * P * P2)

        in_maps.append({"x": x, "y": y, "wblk": wblk,
                        "tm": np.ascontiguousarray(tm), "ones": ones})
    return in_maps


def kernel(**inputs):
    global last_results
    nc = _build()
    in_maps = _host_prep(**inputs)
    trace = bool(os.environ.get("KERNEL_TRACE"))
    res = run_bass_kernel_spmd(nc, in_maps, core_ids=list(range(N_CORES)),
                               trace=trace)
    last_results = res
    att = np.stack([res.results[r]["att"] for r in range(N_CORES)])
    return att.reshape(1, C, L, L)



# revision 14
# speedup vs baseline: 1.6417x; 1.6417x over previous
"""Trainium2 Bass kernel for nn_Cross_attention_2 (sparse_attention).

Math (B=1, C=32, D=36, H=W=48, P=9):
  xc = conv1x1(x, W_img)                   # bias folded into transform
  v  = unfold(xc)                          # (C, L=1024, 81) non-overlapping 9x9 patches
  px = LeakyReLU(v @ A^T + b.rowsum(A))    # A = W2@W1 collapsed
  att[c] = px[c] @ py[c]^T / 81            # (C, 1024, 1024)

Sharding: channels C=32 split across 8 cores (4 each); params replicated
(per-core slices precomputed on host). Each core reads full x, y.

v3 (fp16 + PE sub-array tiling):
  - All SBUF data fp16 (PSUM accumulation fp32); inputs uploaded fp16,
    att written fp16 and upcast on host. End-to-end rel err ~7e-4.
  - Input loads issued as (32, nkd, 2304) APs - outer dim 32 spreads
    descriptors over all 16 DMA engines (the old (4, 32, 2304) transpose
    APs landed on engines 0-3 only and serialized the whole kernel).
  - xcA rows are 32*c + kd (c=0..2) with a 1.0 bias row at 32*c+9: each
    channel's unfold contraction is K=10 at a 32-aligned partition base,
    so channels 0-2's transform matmuls land on disjoint PE row groups
    (tile_position auto-derived from AP bases) and run CONCURRENTLY on
    the 32x32 sub-array grid. Channel 3 lives in xcB at base 0 (row
    group 3 / base 96 is not usable) and time-shares row group 0.
  - conv: 3 accumulating matmuls per pd (K=128/128/33, M=128; the K=33
    pass carries a ones row that writes the bias rows), PSUM rows 0-95
    copied to xcA, rows 96-127 to xcB (both casts to fp16).
  - transform: 9 kw passes x 4 channels (3 concurrent), K=10 (9 kd rows
    + bias row, channel selected by the rhs partition slice); bias
    weights live in the per-channel lhsT bias row at kw=0 (channel 3's
    in the small tmb tile). LeakyReLU = max(0.2z, z) as Act mul + DVE
    max (one instruction cannot read two PSUM operands).
  - att: pxT kept as (81, 1024) fp16; (128, 512) fp32 PSUM tiles cast to
    fp16 on the PSUM->SBUF copy; per-channel output DMA overlaps the
    next channel's compute.
"""

import sys

sys.path.insert(0, "/opt/trn_rl_repo")

import contextlib
import os

import numpy as np

import concourse.bass as bass  # noqa: F401
import concourse.tile as tile
from concourse import bacc, mybir
from concourse.bass_utils import run_bass_kernel_spmd

P = 9
P2 = 81
C = 32
D = 36
HWF = 2304
ND = 4  # pd blocks (D/9)
L = 1024
N_CORES = 8
CPC = 4  # channels per core

F16 = mybir.dt.float16
F32 = mybir.dt.float32

_CACHE = {}
last_results = None  # BassKernelResults of the most recent run (for test.py)

_HW_CHUNKS = [(0, 512), (512, 512), (1024, 512), (1536, 512), (2048, 256)]
_KD_PASSES = [(0, 0, 4), (1, 4, 4), (2, 8, 1)]  # (q, kd0, nkd)


def _build():
    if "nc" in _CACHE:
        return _CACHE["nc"]

    nc = bacc.Bacc("TRN2", target_bir_lowering=False, debug=False,
                   num_devices=N_CORES)
    x_d = nc.dram_tensor("x", (C, D, HWF), F16, kind="ExternalInput").ap()
    y_d = nc.dram_tensor("y", (C, D, HWF), F16, kind="ExternalInput").ap()
    # wblk: (128, 768) conv lhsT for (t in 2) x (pass q in 3), 128 cols each
    wblk_d = nc.dram_tensor("wblk", (128, 768), F16, kind="ExternalInput").ap()
    # tm: (96, 2*9*81) transform weights, rows 32c+kd (+ bias row 32c+9)
    tm_d = nc.dram_tensor("tm", (96, 2 * P * P2), F16,
                          kind="ExternalInput").ap()
    # tmb: (10, 2*81) channel-3 kw=0 weights (A kw-slice + c3 bias row)
    tmb_d = nc.dram_tensor("tmb", (10, 2 * P2), F16,
                           kind="ExternalInput").ap()
    att_d = nc.dram_tensor("att", (CPC, L, L), F16, kind="ExternalOutput").ap()

    with tile.TileContext(nc) as tc:
        with contextlib.ExitStack() as ctx:
            consts = ctx.enter_context(tc.tile_pool(name="consts", bufs=1))
            xbp = ctx.enter_context(tc.tile_pool(name="xb", bufs=2))
            tmpp = ctx.enter_context(tc.tile_pool(name="tmp", bufs=2))
            outp = ctx.enter_context(tc.tile_pool(name="outp", bufs=2))
            cps = ctx.enter_context(tc.tile_pool(name="cps", bufs=2, space="PSUM"))
            tps = ctx.enter_context(tc.tile_pool(name="tps", bufs=1, space="PSUM"))
            aps = ctx.enter_context(tc.tile_pool(name="aps", bufs=2, space="PSUM"))

            wb_sb = consts.tile([128, 768], F16, tag="wb")
            nc.sync.dma_start(out=wb_sb[:, :], in_=wblk_d[:, :])
            tm_sb = consts.tile([96, 2 * P * P2], F16, tag="tm")
            nc.sync.dma_start(out=tm_sb[:, :], in_=tm_d[:, :])
            tm_v = tm_sb.rearrange("p (t kw j) -> p t kw j", t=2, kw=P)
            tmb_sb = consts.tile([10, 2 * P2], F16, tag="tmb")
            nc.sync.dma_start(out=tmb_sb[:, :], in_=tmb_d[:, :])
            tmb_v = tmb_sb.rearrange("p (t j) -> p t j", t=2)

            xca = [consts.tile([96, ND, HWF], F16, tag=f"xca{t}",
                               name=f"xca{t}")
                   for t in range(2)]
            xcb = [consts.tile([32, ND, HWF], F16, tag=f"xcb{t}",
                               name=f"xcb{t}")
                   for t in range(2)]
            px_sb = [[consts.tile([P2, L], F16, tag=f"px{t}{c}",
                                  name=f"px{t}{c}") for c in range(CPC)]
                     for t in range(2)]

            cp_i = [0]

            def psum_copy(dst, src):
                # rotate PSUM->SBUF copies between DVE and Act
                if cp_i[0] % 2 == 0:
                    nc.vector.tensor_copy(out=dst, in_=src)
                else:
                    nc.scalar.copy(out=dst, in_=src)
                cp_i[0] += 1

            def conv_pd(t, pd):
                src = x_d if t == 0 else y_d
                xbs = []
                for q, kd0, nkd in _KD_PASSES:
                    kp = 32 * nkd + (1 if nkd == 1 else 0)
                    xb = xbp.tile([kp, HWF], F16, tag=f"xb{q}", name=f"xb{q}")
                    d0 = pd * P + kd0
                    # (32, nkd, 2304): outer dim 32 -> descriptors spread
                    # across all 16 DMA engines; partition p = c'*nkd + kd_l
                    nc.sync.dma_start(out=xb[0:32 * nkd, :],
                                      in_=src[:, d0: d0 + nkd, :])
                    if nkd == 1:
                        nc.gpsimd.memset(xb[32:33, :], 1.0)  # bias ones row
                    xbs.append(xb)
                for h0, hn in _HW_CHUNKS:
                    ps = cps.tile([128, 512], F32, tag="cps")
                    for q, kd0, nkd in _KD_PASSES:
                        kp = 32 * nkd + (1 if nkd == 1 else 0)
                        lhs = wb_sb[0:kp, (t * 3 + q) * 128:(t * 3 + q + 1) * 128]
                        nc.tensor.matmul(
                            ps[:, :hn], lhs, xbs[q][:, h0: h0 + hn],
                            start=(q == 0), stop=(q == 2))
                    psum_copy(xca[t][:, pd, h0: h0 + hn], ps[0:96, :hn])
                    # channel 3 rows: partition-shifted 32-row copy (DVE
                    # cross-quadrant moves are aligned-verified)
                    nc.vector.tensor_copy(out=xcb[t][:, pd, h0: h0 + hn],
                                          in_=ps[96:128, :hn])

            def transform(t, ch, zp):
                # z_c = sum_kw TM[c,kw].T @ xc[32c:32c+10, pd-pair, kw::9]
                # K=10 at partition base 32c -> per-channel PE row group;
                # channels 1,2 run on row groups 1,2 concurrently with
                # channels 0,3 time-sharing row group 0.
                for kw in range(P):
                    for c in (1, 2, 0, 3):
                        if c < 3:
                            lhs = tm_v[32 * c: 32 * c + 10, t, kw, :]
                            rhs = xca[t][32 * c: 32 * c + 10,
                                         2 * ch: 2 * ch + 2, kw:HWF:P]
                        else:
                            # rows 0-8 of tm (the A kw-slice) are shared
                            # with channel 0; only kw=0 row 9 (bias) differs
                            lhs = (tmb_v[:, t, :] if kw == 0
                                   else tm_v[0:10, t, kw, :])
                            rhs = xcb[t][0:10, 2 * ch: 2 * ch + 2, kw:HWF:P]
                        nc.tensor.matmul(
                            zp[c][:, :], lhs, rhs,
                            start=(kw == 0), stop=(kw == P - 1))
                for c in range(CPC):
                    # LeakyReLU(z) = max(0.2*z, z)
                    zm = tmpp.tile([P2, 512], F32, tag="zm", name="zm")
                    nc.scalar.mul(zm[:, :], zp[c][:, :], 0.2)
                    nc.vector.tensor_tensor(
                        out=px_sb[t][c][:, ch * 512: ch * 512 + 512],
                        in0=zp[c][:, :], in1=zm[:, :],
                        op=mybir.AluOpType.max)

            def att(c):
                for m in range(8):  # l1 chunks of 128
                    ob = outp.tile([128, L], F16, tag="ob")
                    for nch in range(2):  # l2 chunks of 512
                        ap_ = aps.tile([128, 512], F32, tag="aps")
                        nc.tensor.matmul(
                            ap_[:, :],
                            px_sb[0][c][:, m * 128: m * 128 + 128],
                            px_sb[1][c][:, nch * 512: nch * 512 + 512],
                            start=True, stop=True)
                        psum_copy(ob[:, nch * 512: nch * 512 + 512], ap_[:, :])
                    nc.sync.dma_start(
                        out=att_d[c, m * 128: m * 128 + 128, :], in_=ob[:, :])

            zps = [tps.tile([P2, 512], F32, tag=f"tps{c}", name=f"tps{c}")
                   for c in range(CPC)]
            for t in range(2):
                conv_pd(t, 0)
                conv_pd(t, 1)
                transform(t, 0, zps)
                conv_pd(t, 2)
                conv_pd(t, 3)
                transform(t, 1, zps)
                if t == 1:
                    for c in range(CPC):
                        att(c)

    nc.compile()
    _CACHE["nc"] = nc
    return nc


def _host_prep(x, y, W_img, b_img, W_fea, b_fea, W1, W2):
    """Build per-core wblk / tm arrays. Returns in_maps list."""
    x = np.ascontiguousarray(
        np.asarray(x, np.float32).reshape(C, D, HWF).astype(np.float16))
    y = np.ascontiguousarray(
        np.asarray(y, np.float32).reshape(C, D, HWF).astype(np.float16))
    W_img = np.asarray(W_img, np.float32)
    b_img = np.asarray(b_img, np.float32)
    W_fea = np.asarray(W_fea, np.float32)
    b_fea = np.asarray(b_fea, np.float32)
    A = np.asarray(W2, np.float32) @ np.asarray(W1, np.float32)  # (81, 81)
    rowsum = A.sum(axis=1)  # (81,)
    At = np.stack([A / P2, A])  # x-side carries the 1/81

    in_maps = []
    for r in range(N_CORES):
        Wl = [W_img[r * CPC:(r + 1) * CPC, :], W_fea[r * CPC:(r + 1) * CPC, :]]
        bl = [b_img[r * CPC:(r + 1) * CPC], b_fea[r * CPC:(r + 1) * CPC]]

        # conv lhsT: per (t, pass q): 128 cols, col m = 32*o + kd
        # passes q0/q1: rows p = c'*4 + kd_l (kd = kd0 + kd_l)
        # pass q2 (kd=8): rows p = c' (0..31); ones row 32 -> bias cols 32o+9
        wblk = np.zeros((128, 768), np.float32)
        for t in range(2):
            for q, kd0, nkd in _KD_PASSES:
                base = (t * 3 + q) * 128
                for kd_l in range(nkd):
                    rows = slice(kd_l, 32 * nkd, nkd) if nkd > 1 else slice(0, 32)
                    for o in range(CPC):
                        wblk[rows, base + 32 * o + kd0 + kd_l] = Wl[t][o, :]
            for o in range(CPC):
                wblk[32, (t * 3 + 2) * 128 + 32 * o + 9] = 1.0

        # tm[32c+kd, t, kw, j] = At[t, j, kd*9+kw]; bias row 32c+9 at kw=0
        tm = np.zeros((96, 2, P, P2), np.float32)
        bias = np.stack([np.outer(bl[0], rowsum) / P2,
                         np.outer(bl[1], rowsum)])  # (2, 4, 81)
        for c in range(3):
            for kd in range(P):
                # tm[32c+kd, t, kw, j] = At[t, j, kd*9+kw]
                tm[32 * c + kd] = At[:, :, kd * P:(kd + 1) * P].transpose(0, 2, 1)
            tm[32 * c + 9, :, 0, :] = bias[:, c, :]
        tm = tm.reshape(96, 2 * P * P2).astype(np.float16)

        # tmb: channel 3's kw=0 lhsT (A kw=0 slice + its bias row)
        tmb = np.zeros((10, 2, P2), np.float32)
        for kd in range(P):
            tmb[kd] = At[:, :, kd * P]  # kw = 0
        tmb[9] = bias[:, 3, :]
        tmb = tmb.reshape(10, 2 * P2).astype(np.float16)

        in_maps.append({"x": x, "y": y,
                        "wblk": wblk.astype(np.float16),
                        "tm": np.ascontiguousarray(tm),
                        "tmb": np.ascontiguousarray(tmb)})
    return in_maps


def kernel(**inputs):
    global last_results
    nc = _build()
    in_maps = _host_prep(**inputs)
    trace = bool(os.environ.get("KERNEL_TRACE"))
    res = run_bass_kernel_spmd(nc, in_maps, core_ids=list(range(N_CORES)),
                               trace=trace)
    last_results = res
    att = np.stack([res.results[r]["att"] for r in range(N_CORES)])
    return att.reshape(1, C, L, L).astype(np.float32)
